# revision 34
# baseline (speedup 1.0000x reference)
"""Bahdanau-style attention kernel for Trainium2 (8 NeuronCores, data-parallel).

Computes, for each batch b:
    h_proj = hidden @ w_h^T + attn_b                  # [H]
    e_proj = enc[b] @ w_e^T                           # [L, H]
    energy = tanh(h_proj + e_proj)                    # [L, H]
    scores = energy @ v_w                             # [L]
    weights = softmax(scores)                         # [L]
    context[b] = weights @ enc[b]                     # [H]

Sharding: data-parallel over batch B=32 across 8 cores (4 batches/core).
Params are replicated. The softmax max-subtraction is skipped (scores are
bounded by sum|v| <= 32, exp is safe in fp32); the 1/Z normalization is
folded into the final context scaling.

The dominant GEMM (e_proj) runs in fp8 e4m3 with perf_mode=DoubleRow
(2 fp8 weights per PE cell, 2 MACs/cycle -> ~2x the bf16/f32r rate).
enc is pre-scaled by 16 and w_e by 8192 on the host so both fit the
e4m3 grid well clear of the +-240 TRN limit; the 2^-17 descale is folded
into the tanh activation's scale input. The softmax/context path keeps
enc in fp32 (encN), so only e_proj sees fp8 quantization error
(~1e-2 rel, tolerance is 2e-2).

Built on bacc.Bacc so compile() runs the TRN2 wait-splitting passes
(move_matmul_waits_to_ldweights / generate_event_semaphores).
"""

import numpy as np

H = 1024
B = 32
L = 2048
NCORES = 8
BPC = B // NCORES          # batches per core = 4
KC = H // 128              # contraction chunks of 128 = 8
KC2 = H // 256             # DoubleRow contraction chunks of 256 = 4
OC = H // 128              # output-feature chunks = 8
NLT = L // 512             # l-tiles of 512 = 4
NLCH = L // 128            # l-chunks of 128 = 16

ENC_SCALE = 16.0           # enc pre-scale before e4m3 quantization
W_SCALE = 8192.0           # w_e pre-scale before e4m3 quantization
DESCALE = 1.0 / (ENC_SCALE * W_SCALE)   # folded into tanh activation

_CACHED_NC = None


def _build_kernel():
    from contextlib import ExitStack

    import concourse.tile as tile
    from concourse import bacc
    from concourse import mybir
    from concourse.masks import make_identity

    f32 = mybir.dt.float32
    f32r = mybir.dt.float32r
    bf16 = mybir.dt.bfloat16
    fp8 = mybir.dt.float8e4
    AF = mybir.ActivationFunctionType
    DR = mybir.MatmulPerfMode.DoubleRow

    nc = bacc.Bacc("TRN2", target_bir_lowering=False, debug=False,
                   num_devices=NCORES)

    # all inputs host-laid-out so every DMA is contiguous per partition
    encT = nc.dram_tensor("encTr", [BPC, 128, NLT, KC, 512], fp8,
                          kind="ExternalInput").ap()
    encN = nc.dram_tensor("encNr", [BPC, 128, NLT, 4, H], bf16,
                          kind="ExternalInput").ap()
    w_eT = nc.dram_tensor("wer", [128, KC, H], fp8, kind="ExternalInput").ap()
    smallr = nc.dram_tensor("smallr", [128, OC + OC * BPC], f32,
                            kind="ExternalInput").ap()
    ctx_out = nc.dram_tensor("ctx", [BPC, H], f32, kind="ExternalOutput").ap()
    # DRAM bounce buffer used to transpose exp(scores) [1,512] -> [128,4]
    escr = nc.dram_tensor("escr", [BPC, L], bf16).ap()

    with tile.TileContext(nc) as tc, ExitStack() as ctx:
        consts = ctx.enter_context(tc.tile_pool(name="consts", bufs=1))
        encT_pool = ctx.enter_context(tc.tile_pool(name="encT", bufs=4))
        encN_pool = ctx.enter_context(tc.tile_pool(name="encN", bufs=4))
        en_pool = ctx.enter_context(tc.tile_pool(name="energy", bufs=4))
        small = ctx.enter_context(tc.tile_pool(name="small", bufs=2))
        expwT_pool = ctx.enter_context(tc.tile_pool(name="expwT", bufs=2))

        # ---- constants ----
        # tiny consts + first enc slab (chunked for earliest start) on sync;
        # the w_e load rides the scalar queue in parallel.
        small_sb = consts.tile([128, OC + OC * BPC], f32)
        nc.sync.dma_start(out=small_sb, in_=smallr)
        v_sb = small_sb[:, 0:OC]
        # h_proj + attn_b, host-folded: [128, OC, BPC]
        hproj_sb = small_sb[:, OC:].rearrange("p (o b) -> p o b", b=BPC)
        encTs_pre = encT_pool.tile([128, KC, 512], fp8, tag="encTs",
                                   name="encTs_pre")
        nc.scalar.dma_start(out=encTs_pre, in_=encT[0, :, 0])
        we_sb = consts.tile([128, KC, H], fp8)           # w_e^T  [h-part, k, o]
        for k in range(KC):
            eng = nc.scalar if k < KC // 2 else nc.sync
            eng.dma_start(out=we_sb[:, k, :], in_=w_eT[:, k, :])
        ident = consts.tile([128, 128], f32)
        make_identity(nc, ident)
        ident_bf = consts.tile([128, 128], bf16)
        nc.vector.tensor_copy(ident_bf, ident)
        ones_f32 = consts.tile([128, 1], f32)
        nc.vector.memset(ones_f32, 1.0)
        ones_bf = consts.tile([128, 1], bf16)
        nc.vector.tensor_copy(ones_bf, ones_f32)

        with tc.tile_pool(name="pp_pro", bufs=1, space="PSUM") as pp_pro:
            # warm the PE HAM while the weight DMAs stream (bf16: 1-pass MMs)
            pwarm = pp_pro.tile([128, 128], f32, tag="pwarm")
            for w in range(24):
                nc.tensor.matmul(pwarm, ident_bf, ident_bf, start=True,
                                 stop=True, skip_group_check=True)

        pp_e = ctx.enter_context(tc.tile_pool(name="pp_e", bufs=4, space="PSUM"))
        pp_s = ctx.enter_context(tc.tile_pool(name="pp_s", bufs=1, space="PSUM"))
        pp_c = ctx.enter_context(tc.tile_pool(name="pp_c", bufs=3, space="PSUM"))

        # ---- main pipeline: flat stream of l-slabs across batches ----
        # Stages deferred so no PE instruction ever waits on a same-slab
        # producer chain:
        #   stage1(s): enc DMAs + DR matmuls + tanh + v-weighted accumulate
        #   stage2(s): scores partition-reduce (ones-MM) + exp + transpose
        #              bounce — emitted at the TOP of slab s+1 (the ones-MM
        #              leads the PE stream so exp/bounce start early)
        #   stage3(s): context matmuls (h 0:512 on PE) + context STT
        #              accumulate (h 512:1024 on GpSimd) — emitted during
        #              slab s+2
        #   finalize(b): one further slab behind, so the PE's Z-scale ones-MM
        #              never waits on the GpSimd STT chain
        S = BPC * NLT
        state = {}

        def stage1(s):
            b, lt = divmod(s, NLT)
            if lt == 0:
                state[b] = {
                    "expwT": expwT_pool.tile([128, NLCH], bf16, tag="expwT",
                                             name=f"expwT{b}"),
                    "pcs": None,
                    "zacc": small.tile([1, NLT], f32, tag="zacc",
                                       name=f"zacc{b}"),
                }
            st = state[b]
            if s == 0:
                encTs = encTs_pre
            else:
                encTs = encT_pool.tile([128, KC, 512], fp8, tag="encTs")
                nc.sync.dma_start(out=encTs, in_=encT[b, :, lt])
            # encN prefetch: scalar queue, issued at the top of the iteration
            # (right after stage2's exp) so the transfer has ~2 slabs of lead.
            encNs = encN_pool.tile([128, 4, H], bf16, tag="encNs",
                                   name=f"encNs{b}_{lt}")
            nc.scalar.dma_start(out=encNs, in_=encN[b, :, lt])
            st[f"encNs{lt}"] = encNs
            acc = en_pool.tile([128, 512], bf16, tag="acc", name=f"acc{s}")
            for o in range(OC):
                pe = pp_e.tile([128, 512], f32, tag="pe")
                for k2 in range(KC2):
                    nc.tensor.matmul(
                        pe,
                        we_sb[:, 2 * k2:2 * k2 + 2, o * 128:(o + 1) * 128],
                        encTs[:, 2 * k2:2 * k2 + 2, :],
                        start=(k2 == 0), stop=(k2 == KC2 - 1),
                        perf_mode=DR,
                    )
                en = en_pool.tile([128, 512], f32, tag="en")
                nc.scalar.activation(en, pe, AF.Tanh, scale=DESCALE,
                                     bias=hproj_sb[:, o, b:b + 1])
                # accumulate v-weighted energy on DVE (partition-wise)
                if o == 0:
                    nc.vector.tensor_scalar_mul(acc, en, v_sb[:, 0:1])
                else:
                    nc.vector.scalar_tensor_tensor(
                        out=acc, in0=en, scalar=v_sb[:, o:o + 1], in1=acc,
                        op0=mybir.AluOpType.mult, op1=mybir.AluOpType.add)
            st[f"acc{lt}"] = acc

        def stage2(s):
            b, lt = divmod(s, NLT)
            st = state[b]
            acc = st.pop(f"acc{lt}")
            # partition reduction of acc via ones-matmul (bf16: full rate)
            psum_sc = pp_s.tile([1, 512], f32, tag="psc")
            nc.tensor.matmul(psum_sc, ones_bf, acc, start=True, stop=True)
            # exp (no max subtraction; scores bounded), Z-part for free
            expw = small.tile([1, 512], bf16, tag="expw")
            nc.scalar.activation(expw, psum_sc, AF.Exp,
                                 accum_out=st["zacc"][:, lt:lt + 1])
            # transpose exp(scores) into [l-part, chunk] layout via DRAM.
            # Rides the sync queue: exp lands early in the iteration (stage2
            # leads the PE/ACT streams), and the encT prefetch behind it has
            # a full slab of slack. The gpsimd queue stalled these behind
            # unrelated ring-reuse waits.
            nc.sync.dma_start(
                out=escr[b:b + 1, lt * 512:(lt + 1) * 512], in_=expw)
            nc.sync.dma_start(
                out=st["expwT"][:, lt * 4:(lt + 1) * 4],
                in_=escr[b, lt * 512:(lt + 1) * 512]
                .rearrange("(c p) -> p c", p=128),
            )

        def stage3(s):
            b, lt = divmod(s, NLT)
            st = state[b]
            encNs = st.pop(f"encNs{lt}")
            if st["pcs"] is None:
                st["pcs"] = [pp_c.tile([1, 512], f32, tag="pc",
                                       name=f"pc{b}_{i}") for i in range(2)]
            for j in range(4):
                lc = lt * 4 + j
                for half in range(2):
                    nc.tensor.matmul(
                        st["pcs"][half],
                        st["expwT"][:, lc:lc + 1],
                        encNs[:, j, half * 512:(half + 1) * 512],
                        start=(lc == 0), stop=(lc == NLCH - 1),
                    )
            if lt == NLT - 1:
                finalize(b)

        def finalize(b):
            st = state.pop(b)
            zs = small.tile([1, 1], f32, tag="zs", name=f"zs{b}")
            nc.vector.reduce_sum(zs, st["zacc"], axis=mybir.AxisListType.X)
            rz = small.tile([1, 1], f32, tag="rz", name=f"rz{b}")
            nc.vector.reciprocal(rz, zs)
            ctx_sb = small.tile([1, H], f32, tag="ctx", name=f"ctx{b}")
            for half in range(2):
                nc.vector.tensor_scalar_mul(
                    ctx_sb[:, half * 512:(half + 1) * 512],
                    st["pcs"][half], rz)
            eng = nc.sync if b == BPC - 1 else nc.gpsimd
            eng.dma_start(out=ctx_out[b:b + 1, :], in_=ctx_sb)

        for s in range(S):
            stage1(s)
            if s >= 1:
                stage2(s - 1)
            if s >= 2:
                stage3(s - 2)
        stage2(S - 1)
        stage3(S - 2)
        stage3(S - 1)

    nc.compile()
    return nc


def _get_nc():
    global _CACHED_NC
    if _CACHED_NC is None:
        _CACHED_NC = _build_kernel()
    return _CACHED_NC


def _make_in_maps(hidden, encoder_outputs, attn_w, attn_b, v_w):
    import ml_dtypes

    e4m3 = ml_dtypes.float8_e4m3

    hidden = np.asarray(hidden, dtype=np.float32)
    encoder_outputs = np.asarray(encoder_outputs, dtype=np.float32)
    attn_w = np.asarray(attn_w, dtype=np.float32)
    attn_b = np.asarray(attn_b, dtype=np.float32)
    v_w = np.asarray(v_w, dtype=np.float32)

    wer32 = np.ascontiguousarray(
        attn_w[:, H:].T.reshape(KC, 128, H).transpose(1, 0, 2))
    wer = np.clip(wer32 * W_SCALE, -240.0, 240.0).astype(e4m3)
    # fold the tiny h_proj = hidden @ w_h^T + b into a per-core bias input
    hproj_pb = hidden @ attn_w[:, :H].T + attn_b     # [B, H]

    enc8_full = np.clip(encoder_outputs * ENC_SCALE, -240.0, 240.0).astype(e4m3)

    in_maps = []
    for c in range(NCORES):
        sl = slice(c * BPC, (c + 1) * BPC)
        enc = encoder_outputs[sl]                       # [BPC, L, H]
        # encTr[b, p, lt, k, l] = q(enc[b, lt*512 + l, k*128 + p] * 16)
        encTr = np.ascontiguousarray(
            enc8_full[sl].reshape(BPC, NLT, 512, KC, 128)
            .transpose(0, 4, 1, 3, 2))
        # encNr[b, p, lt, j, h] = enc[b, lt*512 + j*128 + p, h]  (bf16)
        encNr = np.ascontiguousarray(
            enc.reshape(BPC, NLT, 4, 128, H).transpose(0, 3, 1, 2, 4)
            .astype(ml_dtypes.bfloat16))
        # smallr: [v chunks | h_proj+b chunks]  (hp[p, o, b] layout)
        hp = hproj_pb[sl].T.reshape(OC, 128, BPC).transpose(1, 0, 2)
        smallr = np.concatenate([
            v_w.reshape(OC, 128).T,
            hp.reshape(128, OC * BPC),
        ], axis=1)
        in_maps.append({
            "encTr": encTr,
            "encNr": encNr,
            "wer": wer,
            "smallr": np.ascontiguousarray(smallr),
        })
    return in_maps


def kernel(hidden, encoder_outputs, attn_w, attn_b, v_w):
    from concourse.bass_utils import run_bass_kernel_spmd

    in_maps = _make_in_maps(hidden, encoder_outputs, attn_w, attn_b, v_w)
    nc = _get_nc()
    res = run_bass_kernel_spmd(nc, in_maps, list(range(NCORES)))
    out = np.concatenate([res.results[c]["ctx"] for c in range(NCORES)], axis=0)
    return out.astype(np.float32)



# revision 36
# speedup vs baseline: 1.2218x; 1.2218x over previous
"""Bahdanau-style attention kernel for Trainium2 (8 NeuronCores, data-parallel).

Computes, for each batch b:
    h_proj = hidden @ w_h^T + attn_b                  # [H]
    e_proj = enc[b] @ w_e^T                           # [L, H]
    energy = tanh(h_proj + e_proj)                    # [L, H]
    scores = energy @ v_w                             # [L]
    weights = softmax(scores)                         # [L]
    context[b] = weights @ enc[b]                     # [H]

Sharding: data-parallel over batch B=32 across 8 cores (4 batches/core).
Params are replicated. The softmax max-subtraction is skipped (scores are
bounded by sum|v| <= 32, exp is safe in fp32); the 1/Z normalization is
folded into the final context scaling.

The dominant GEMM (e_proj) runs in fp8 e4m3 with perf_mode=DoubleRow
(2 fp8 weights per PE cell, 2 MACs/cycle -> ~2x the bf16/f32r rate).
enc is pre-scaled by 16 and w_e by 8192 on the host so both fit the
e4m3 grid well clear of the +-240 TRN limit; the 2^-17 descale is folded
into the tanh activation's scale input. The softmax/context path keeps
enc in fp32 (encN), so only e_proj sees fp8 quantization error
(~1e-2 rel, tolerance is 2e-2).

Built on bacc.Bacc so compile() runs the TRN2 wait-splitting passes
(move_matmul_waits_to_ldweights / generate_event_semaphores).
"""

import numpy as np

H = 1024
B = 32
L = 2048
NCORES = 8
BPC = B // NCORES          # batches per core = 4
KC = H // 128              # contraction chunks of 128 = 8
KC2 = H // 256             # DoubleRow contraction chunks of 256 = 4
OC = H // 128              # output-feature chunks = 8
NLT = L // 512             # l-tiles of 512 = 4
NLCH = L // 128            # l-chunks of 128 = 16

ENC_SCALE = 16.0           # enc pre-scale before e4m3 quantization
W_SCALE = 8192.0           # w_e pre-scale before e4m3 quantization
DESCALE = 1.0 / (ENC_SCALE * W_SCALE)   # folded into tanh activation

_CACHED_NC = None


def _build_kernel():
    from contextlib import ExitStack

    import concourse.tile as tile
    from concourse import bacc
    from concourse import mybir
    from concourse.masks import make_identity

    f32 = mybir.dt.float32
    f32r = mybir.dt.float32r
    bf16 = mybir.dt.bfloat16
    fp8 = mybir.dt.float8e4
    AF = mybir.ActivationFunctionType
    DR = mybir.MatmulPerfMode.DoubleRow

    nc = bacc.Bacc("TRN2", target_bir_lowering=False, debug=False,
                   num_devices=NCORES)

    # all inputs host-laid-out so every DMA is contiguous per partition
    encT = nc.dram_tensor("encTr", [BPC, 128, NLT, KC, 512], fp8,
                          kind="ExternalInput").ap()
    encN = nc.dram_tensor("encNr", [BPC, 128, NLT, 4, H], bf16,
                          kind="ExternalInput").ap()
    w_eT = nc.dram_tensor("wer", [128, KC, H], fp8, kind="ExternalInput").ap()
    smallr = nc.dram_tensor("smallr", [128, OC + OC * BPC], f32,
                            kind="ExternalInput").ap()
    ctx_out = nc.dram_tensor("ctx", [BPC, H], f32, kind="ExternalOutput").ap()
    # DRAM bounce buffer used to transpose exp(scores) [1,512] -> [128,4]
    escr = nc.dram_tensor("escr", [BPC, L], bf16).ap()

    with tile.TileContext(nc) as tc, ExitStack() as ctx:
        consts = ctx.enter_context(tc.tile_pool(name="consts", bufs=1))
        encT_pool = ctx.enter_context(tc.tile_pool(name="encT", bufs=4))
        encN_pool = ctx.enter_context(tc.tile_pool(name="encN", bufs=4))
        en_pool = ctx.enter_context(tc.tile_pool(name="energy", bufs=4))
        small = ctx.enter_context(tc.tile_pool(name="small", bufs=2))
        expwT_pool = ctx.enter_context(tc.tile_pool(name="expwT", bufs=2))

        # ---- constants ----
        # tiny consts + first enc slab (chunked for earliest start) on sync;
        # the w_e load rides the scalar queue in parallel.
        small_sb = consts.tile([128, OC + OC * BPC], f32)
        nc.sync.dma_start(out=small_sb, in_=smallr)
        v_sb = small_sb[:, 0:OC]
        # h_proj + attn_b, host-folded: [128, OC, BPC]
        hproj_sb = small_sb[:, OC:].rearrange("p (o b) -> p o b", b=BPC)
        encTs_pre = encT_pool.tile([128, KC, 512], fp8, tag="encTs",
                                   name="encTs_pre")
        nc.scalar.dma_start(out=encTs_pre, in_=encT[0, :, 0])
        we_sb = consts.tile([128, KC, H], fp8)           # w_e^T  [h-part, k, o]
        for k in range(KC):
            eng = nc.scalar if k < KC // 2 else nc.sync
            eng.dma_start(out=we_sb[:, k, :], in_=w_eT[:, k, :])
        ident = consts.tile([128, 128], f32)
        make_identity(nc, ident)
        ident_bf = consts.tile([128, 128], bf16)
        nc.vector.tensor_copy(ident_bf, ident)
        ones_f32 = consts.tile([128, 1], f32)
        nc.vector.memset(ones_f32, 1.0)
        ones_bf = consts.tile([128, 1], bf16)
        nc.vector.tensor_copy(ones_bf, ones_f32)

        with tc.tile_pool(name="pp_pro", bufs=1, space="PSUM") as pp_pro:
            # warm the PE HAM while the weight DMAs stream (bf16: 1-pass MMs)
            pwarm = pp_pro.tile([128, 128], f32, tag="pwarm")
            for w in range(24):
                nc.tensor.matmul(pwarm, ident_bf, ident_bf, start=True,
                                 stop=True, skip_group_check=True)

        # 5 banks for the energy GEMM: the tanh drain lags the GEMM by ~1
        # group, and with only 4 banks the first matmul of each slab stalled
        # ~0.8us on the ACT semaphore. pp_c's ring needs only 2 (pc/pc1
        # alternate and are each freed in finalize before reuse).
        pp_e = ctx.enter_context(tc.tile_pool(name="pp_e", bufs=5, space="PSUM"))
        pp_s = ctx.enter_context(tc.tile_pool(name="pp_s", bufs=1, space="PSUM"))
        pp_c = ctx.enter_context(tc.tile_pool(name="pp_c", bufs=2, space="PSUM"))

        # ---- main pipeline: flat stream of l-slabs across batches ----
        # Stages deferred so no PE instruction ever waits on a same-slab
        # producer chain:
        #   stage1(s): enc DMAs + DR matmuls + tanh + v-weighted accumulate
        #   stage2(s): scores partition-reduce (ones-MM) + exp + transpose
        #              bounce — emitted at the TOP of slab s+1 (the ones-MM
        #              leads the PE stream so exp/bounce start early)
        #   stage3(s): context matmuls (h 0:512 on PE) + context STT
        #              accumulate (h 512:1024 on GpSimd) — emitted during
        #              slab s+2
        #   finalize(b): one further slab behind, so the PE's Z-scale ones-MM
        #              never waits on the GpSimd STT chain
        S = BPC * NLT
        state = {}

        def stage1(s):
            b, lt = divmod(s, NLT)
            if lt == 0:
                state[b] = {
                    "expwT": expwT_pool.tile([128, NLCH], bf16, tag="expwT",
                                             name=f"expwT{b}"),
                    "pcs": None,
                    "zacc": small.tile([1, NLT], f32, tag="zacc",
                                       name=f"zacc{b}"),
                }
            st = state[b]
            if s == 0:
                encTs = encTs_pre
            else:
                encTs = encT_pool.tile([128, KC, 512], fp8, tag="encTs")
                nc.sync.dma_start(out=encTs, in_=encT[b, :, lt])
            # encN prefetch: scalar queue, issued at the top of the iteration
            # (right after stage2's exp) so the transfer has ~2 slabs of lead.
            encNs = encN_pool.tile([128, 4, H], bf16, tag="encNs",
                                   name=f"encNs{b}_{lt}")
            nc.scalar.dma_start(out=encNs, in_=encN[b, :, lt])
            st[f"encNs{lt}"] = encNs
            acc = en_pool.tile([128, 512], bf16, tag="acc", name=f"acc{s}")
            for o in range(OC):
                pe = pp_e.tile([128, 512], f32, tag="pe")
                for k2 in range(KC2):
                    nc.tensor.matmul(
                        pe,
                        we_sb[:, 2 * k2:2 * k2 + 2, o * 128:(o + 1) * 128],
                        encTs[:, 2 * k2:2 * k2 + 2, :],
                        start=(k2 == 0), stop=(k2 == KC2 - 1),
                        perf_mode=DR,
                    )
                en = en_pool.tile([128, 512], f32, tag="en")
                nc.scalar.activation(en, pe, AF.Tanh, scale=DESCALE,
                                     bias=hproj_sb[:, o, b:b + 1])
                # accumulate v-weighted energy on DVE (partition-wise)
                if o == 0:
                    nc.vector.tensor_scalar_mul(acc, en, v_sb[:, 0:1])
                else:
                    nc.vector.scalar_tensor_tensor(
                        out=acc, in0=en, scalar=v_sb[:, o:o + 1], in1=acc,
                        op0=mybir.AluOpType.mult, op1=mybir.AluOpType.add)
            st[f"acc{lt}"] = acc

        def stage2(s):
            b, lt = divmod(s, NLT)
            st = state[b]
            acc = st.pop(f"acc{lt}")
            # partition reduction of acc via ones-matmul (bf16: full rate)
            psum_sc = pp_s.tile([1, 512], f32, tag="psc")
            nc.tensor.matmul(psum_sc, ones_bf, acc, start=True, stop=True)
            # exp (no max subtraction; scores bounded), Z-part for free
            expw = small.tile([1, 512], bf16, tag="expw")
            nc.scalar.activation(expw, psum_sc, AF.Exp,
                                 accum_out=st["zacc"][:, lt:lt + 1])
            # transpose exp(scores) into [l-part, chunk] layout via DRAM.
            # Rides the sync queue: exp lands early in the iteration (stage2
            # leads the PE/ACT streams), and the encT prefetch behind it has
            # a full slab of slack. The gpsimd queue stalled these behind
            # unrelated ring-reuse waits.
            nc.sync.dma_start(
                out=escr[b:b + 1, lt * 512:(lt + 1) * 512], in_=expw)
            nc.sync.dma_start(
                out=st["expwT"][:, lt * 4:(lt + 1) * 4],
                in_=escr[b, lt * 512:(lt + 1) * 512]
                .rearrange("(c p) -> p c", p=128),
            )

        def stage3(s):
            b, lt = divmod(s, NLT)
            st = state[b]
            encNs = st.pop(f"encNs{lt}")
            if st["pcs"] is None:
                st["pcs"] = [pp_c.tile([1, 512], f32, tag="pc",
                                       name=f"pc{b}_{i}") for i in range(2)]
            for j in range(4):
                lc = lt * 4 + j
                for half in range(2):
                    nc.tensor.matmul(
                        st["pcs"][half],
                        st["expwT"][:, lc:lc + 1],
                        encNs[:, j, half * 512:(half + 1) * 512],
                        start=(lc == 0), stop=(lc == NLCH - 1),
                    )
            if lt == NLT - 1:
                finalize(b)

        def finalize(b):
            st = state.pop(b)
            zs = small.tile([1, 1], f32, tag="zs", name=f"zs{b}")
            nc.vector.reduce_sum(zs, st["zacc"], axis=mybir.AxisListType.X)
            rz = small.tile([1, 1], f32, tag="rz", name=f"rz{b}")
            nc.vector.reciprocal(rz, zs)
            ctx_sb = small.tile([1, H], f32, tag="ctx", name=f"ctx{b}")
            for half in range(2):
                nc.vector.tensor_scalar_mul(
                    ctx_sb[:, half * 512:(half + 1) * 512],
                    st["pcs"][half], rz)
            eng = nc.sync if b == BPC - 1 else nc.gpsimd
            eng.dma_start(out=ctx_out[b:b + 1, :], in_=ctx_sb)

        for s in range(S):
            if s >= 1:
                stage2(s - 1)
            stage1(s)
            if s >= 2:
                stage3(s - 2)
        stage2(S - 1)
        stage3(S - 2)
        stage3(S - 1)

    nc.compile()
    return nc


def _get_nc():
    global _CACHED_NC
    if _CACHED_NC is None:
        _CACHED_NC = _build_kernel()
    return _CACHED_NC


def _make_in_maps(hidden, encoder_outputs, attn_w, attn_b, v_w):
    import ml_dtypes

    e4m3 = ml_dtypes.float8_e4m3

    hidden = np.asarray(hidden, dtype=np.float32)
    encoder_outputs = np.asarray(encoder_outputs, dtype=np.float32)
    attn_w = np.asarray(attn_w, dtype=np.float32)
    attn_b = np.asarray(attn_b, dtype=np.float32)
    v_w = np.asarray(v_w, dtype=np.float32)

    wer32 = np.ascontiguousarray(
        attn_w[:, H:].T.reshape(KC, 128, H).transpose(1, 0, 2))
    wer = np.clip(wer32 * W_SCALE, -240.0, 240.0).astype(e4m3)
    # fold the tiny h_proj = hidden @ w_h^T + b into a per-core bias input
    hproj_pb = hidden @ attn_w[:, :H].T + attn_b     # [B, H]

    enc8_full = np.clip(encoder_outputs * ENC_SCALE, -240.0, 240.0).astype(e4m3)

    in_maps = []
    for c in range(NCORES):
        sl = slice(c * BPC, (c + 1) * BPC)
        enc = encoder_outputs[sl]                       # [BPC, L, H]
        # encTr[b, p, lt, k, l] = q(enc[b, lt*512 + l, k*128 + p] * 16)
        encTr = np.ascontiguousarray(
            enc8_full[sl].reshape(BPC, NLT, 512, KC, 128)
            .transpose(0, 4, 1, 3, 2))
        # encNr[b, p, lt, j, h] = enc[b, lt*512 + j*128 + p, h]  (bf16)
        encNr = np.ascontiguousarray(
            enc.reshape(BPC, NLT, 4, 128, H).transpose(0, 3, 1, 2, 4)
            .astype(ml_dtypes.bfloat16))
        # smallr: [v chunks | h_proj+b chunks]  (hp[p, o, b] layout)
        hp = hproj_pb[sl].T.reshape(OC, 128, BPC).transpose(1, 0, 2)
        smallr = np.concatenate([
            v_w.reshape(OC, 128).T,
            hp.reshape(128, OC * BPC),
        ], axis=1)
        in_maps.append({
            "encTr": encTr,
            "encNr": encNr,
            "wer": wer,
            "smallr": np.ascontiguousarray(smallr),
        })
    return in_maps


def kernel(hidden, encoder_outputs, attn_w, attn_b, v_w):
    from concourse.bass_utils import run_bass_kernel_spmd

    in_maps = _make_in_maps(hidden, encoder_outputs, attn_w, attn_b, v_w)
    nc = _get_nc()
    res = run_bass_kernel_spmd(nc, in_maps, list(range(NCORES)))
    out = np.concatenate([res.results[c]["ctx"] for c in range(NCORES)], axis=0)
    return out.astype(np.float32)

